# revision 10
# baseline (speedup 1.0000x reference)
"""Trainium2 Bass kernel for the attention-encoder (Bahdanau input attention
+ LSTM cell, T-step recurrence).

Math (per batch row b):
    r2 = einsum('tn,tu->nu', x[b], Ue)                 # [N, T], loop-invariant
    per step t:
        r1 = concat(h, s) @ We                         # [T]
        e[n] = sum_t' ve[t'] * tanh(r1[t'] + r2[n,t']) # [N]
        alpha = softmax_n(e)
        z = x_t @ Wk + h @ Wr + b ; LSTM update (keras gate order i,f,g,o)
        out[b, t, :] = alpha * x[b, t, :]

Strategy: pure data parallelism, batch 512 -> 64 per core on 8 cores.
On-chip layout keeps t' on partitions for the big pass:
    r2T [t'(2x128 part), b, n]  (bf16)
    per step: DVE tensor_scalar adds r1[b,t'] (per-partition scalar),
    ACT does one big tanh per chunk, PE contracts t' against a
    per-b "selector" stationary (col b = ve-half) accumulating
    e into PSUM[b, n] -- natural layout for the free-axis softmax.
LSTM runs transposed (zT = [Wk;Wr]^T @ [x_tT; hT]) with sigmoid
computed as 0.5*tanh(0.5x)+0.5 so ACT stays in the exp/tanh table set.
"""

import numpy as np
import ml_dtypes
from contextlib import ExitStack

import concourse.bass as bass
import concourse.bacc as bacc
import concourse.tile as tile
from concourse import mybir
from concourse.bass_utils import run_bass_kernel_spmd

B, T, N, M = 512, 256, 128, 256
NCORES = 8
BL = B // NCORES  # 64 batch rows per core
M4 = 4 * M        # 1024

BF16 = mybir.dt.bfloat16
F32 = mybir.dt.float32
TANH = mybir.ActivationFunctionType.Tanh
EXP = mybir.ActivationFunctionType.Exp
ADD = mybir.AluOpType.add
MULT = mybir.AluOpType.mult

BCHUNK = 32             # b-rows per attention chunk (free = BCHUNK*N = 4096)
NCHUNK = BL // BCHUNK   # 2 chunks per t'-half


# blob free-dim offsets (all [128, *] bf16, packed on host by _marshal)
OFF_XT = 0                       # x_tmaj  [p, 2, BL, N]
OFF_UE = OFF_XT + 2 * BL * N     # Ue      [p, 2, T]
OFF_WE = OFF_UE + 2 * T          # We      [p, 4, T]
OFF_WC = OFF_WE + 4 * T          # Wc      [p, 3, M4]
OFF_VS = OFF_WC + 3 * M4         # vsel    [p, 2, BL, BL]
BLOB_F = OFF_VS + 2 * BL * BL


def build_nc(t_steps: int = T) -> bass.Bass:
    nc = bacc.Bacc(None)

    x_p = nc.declare_dram_parameter("x_b", [BL, T, N], BF16, isOutput=False)
    xn_p = nc.declare_dram_parameter("x_n", [T, N, BL], BF16, isOutput=False)
    blob_p = nc.declare_dram_parameter("blob", [128, BLOB_F], BF16, isOutput=False)
    hT_p = nc.declare_dram_parameter("hT0", [2, 128, BL], BF16, isOutput=False)
    sT_p = nc.declare_dram_parameter("sT0", [2, 128, BL], BF16, isOutput=False)
    be_p = nc.declare_dram_parameter("Be_w", [128, 8], F32, isOutput=False)
    out_p = nc.declare_dram_parameter("out", [BL, T, N], F32, isOutput=True)

    with tile.TileContext(nc) as tc, ExitStack() as ctx:
        singles = ctx.enter_context(tc.tile_pool(name="singles", bufs=1))

        # ---- resident tensors -------------------------------------------
        blob = singles.tile([128, BLOB_F], BF16)
        r2T = singles.tile([128, 2, BL, N], BF16)      # r2[t', b, n]
        be_s = singles.tile([128, 8], F32)             # gate bias (pre-scaled)
        h_bf = singles.tile([128, 2, BL], BF16)        # h^T state
        s_bf = singles.tile([128, 2, BL], BF16)        # s^T state

        x_tmaj = blob[:, OFF_XT:OFF_UE].rearrange(
            "p (h b n) -> p h b n", h=2, b=BL)
        ue_s = blob[:, OFF_UE:OFF_WE].rearrange("p (h t) -> p h t", h=2)
        we_s = blob[:, OFF_WE:OFF_WC].rearrange("p (j t) -> p j t", j=4)
        wc_s = blob[:, OFF_WC:OFF_VS].rearrange("p (j m) -> p j m", j=3)
        vs_s = blob[:, OFF_VS:BLOB_F].rearrange(
            "p (h b m) -> p h b m", h=2, b=BL)

        nc.sync.dma_start(out=blob, in_=blob_p[:])
        nc.sync.dma_start(out=be_s, in_=be_p[:])
        nc.sync.dma_start(out=h_bf, in_=hT_p.rearrange("h p b -> p h b"))
        nc.sync.dma_start(out=s_bf, in_=sT_p.rearrange("h p b -> p h b"))

        # ---- precompute r2T = Ue^T-contraction over t --------------------
        with tc.tile_pool(name="pre_ps", bufs=8, space="PSUM") as pre_ps:
            for c in range(2):          # t'-half (output partitions)
                for b in range(BL):
                    r2p = pre_ps.tile([128, N], F32, tag="r2p")
                    for k in range(2):  # contraction half
                        nc.tensor.matmul(
                            r2p,
                            lhsT=ue_s[:, k, c * 128:(c + 1) * 128],
                            rhs=x_tmaj[:, k, b, :],
                            start=(k == 0),
                            stop=(k == 1),
                        )
                    if b % 2 == 0:
                        nc.vector.tensor_copy(r2T[:, c, b, :], r2p)
                    else:
                        nc.scalar.copy(r2T[:, c, b, :], r2p)

        # ---- per-step pools ---------------------------------------------
        work = ctx.enter_context(tc.tile_pool(name="work", bufs=3))
        gate_pool = ctx.enter_context(tc.tile_pool(name="gates", bufs=2))
        ps_z = ctx.enter_context(tc.tile_pool(name="ps_z", bufs=2, space="PSUM"))
        ps_r1 = ctx.enter_context(tc.tile_pool(name="ps_r1", bufs=2, space="PSUM"))
        ps_e = ctx.enter_context(tc.tile_pool(name="ps_e", bufs=2, space="PSUM"))
        xfeed = ctx.enter_context(tc.tile_pool(name="xfeed", bufs=3))
        opool = ctx.enter_context(tc.tile_pool(name="opool", bufs=3))

        def fetch_x(t):
            x_t_sb = xfeed.tile([BL, N], BF16, tag="x_t")
            nc.sync.dma_start(out=x_t_sb, in_=x_p[:, t, :])
            x_tT_sb = xfeed.tile([128, BL], BF16, tag="x_tT")
            nc.sync.dma_start(out=x_tT_sb, in_=xn_p[t])
            return x_t_sb, x_tT_sb

        x_feed = fetch_x(0)

        for t in range(t_steps):
            x_t_sb, x_tT_sb = x_feed
            if t + 1 < t_steps:
                x_feed = fetch_x(t + 1)

            # ---- r1^T = We^T @ [h; s]  -> [t'(2x128), b] ----------------
            r1_ps = ps_r1.tile([128, 2, BL], F32, tag="r1ps")
            for c in range(2):
                for j in range(4):
                    rhs = h_bf[:, j, :] if j < 2 else s_bf[:, j - 2, :]
                    nc.tensor.matmul(
                        r1_ps[:, c, :],
                        lhsT=we_s[:, j, c * 128:(c + 1) * 128],
                        rhs=rhs,
                        start=(j == 0),
                        stop=(j == 3),
                    )
            r1_sb = work.tile([128, 2, BL], F32, tag="r1sb")
            nc.vector.tensor_copy(r1_sb, r1_ps)

            # ---- z^T = [Wk;Wr]^T @ [x_t^T; h^T] -> [m4(8x128), b] -------
            z_ps = ps_z.tile([128, 8, BL], F32, tag="zps")
            for m in range(8):
                for j in range(3):
                    rhs = x_tT_sb if j == 0 else h_bf[:, j - 1, :]
                    nc.tensor.matmul(
                        z_ps[:, m, :],
                        lhsT=wc_s[:, j, m * 128:(m + 1) * 128],
                        rhs=rhs,
                        start=(j == 0),
                        stop=(j == 2),
                    )

            # ---- gates (sigmoid via tanh, all in exp/tanh table set) ----
            # tiles: 0,1 = i; 2,3 = f; 4,5 = g; 6,7 = o
            t_i = gate_pool.tile([128, 2, BL], BF16, tag="ti")
            t_f = gate_pool.tile([128, 2, BL], BF16, tag="tf")
            t_g = gate_pool.tile([128, 2, BL], BF16, tag="tg")
            t_o = gate_pool.tile([128, 2, BL], BF16, tag="to")
            for u in range(2):
                nc.scalar.activation(t_i[:, u, :], z_ps[:, 0 + u, :], TANH,
                                     bias=be_s[:, 0 + u:1 + u], scale=0.5)
                nc.scalar.activation(t_f[:, u, :], z_ps[:, 2 + u, :], TANH,
                                     bias=be_s[:, 2 + u:3 + u], scale=0.5)
                nc.scalar.activation(t_g[:, u, :], z_ps[:, 4 + u, :], TANH,
                                     bias=be_s[:, 4 + u:5 + u], scale=1.0)
                nc.scalar.activation(t_o[:, u, :], z_ps[:, 6 + u, :], TANH,
                                     bias=be_s[:, 6 + u:7 + u], scale=0.5)

            tanh_s = gate_pool.tile([128, 2, BL], BF16, tag="tanhs")
            for u in range(2):
                fp = gate_pool.tile([128, BL], BF16, tag="fp")
                nc.vector.tensor_scalar(out=fp, in0=t_f[:, u, :],
                                        scalar1=0.5, scalar2=0.5,
                                        op0=MULT, op1=ADD)
                fs = gate_pool.tile([128, BL], BF16, tag="fs")
                nc.vector.tensor_mul(fs, fp, s_bf[:, u, :])
                ip = gate_pool.tile([128, BL], BF16, tag="ip")
                nc.vector.tensor_scalar(out=ip, in0=t_i[:, u, :],
                                        scalar1=0.5, scalar2=0.5,
                                        op0=MULT, op1=ADD)
                ig = gate_pool.tile([128, BL], BF16, tag="ig")
                nc.vector.tensor_mul(ig, ip, t_g[:, u, :])
                nc.vector.tensor_add(s_bf[:, u, :], fs, ig)
                nc.scalar.activation(tanh_s[:, u, :], s_bf[:, u, :], TANH)
                op = gate_pool.tile([128, BL], BF16, tag="op")
                nc.vector.tensor_scalar(out=op, in0=t_o[:, u, :],
                                        scalar1=0.5, scalar2=0.5,
                                        op0=MULT, op1=ADD)
                nc.vector.tensor_mul(h_bf[:, u, :], op, tanh_s[:, u, :])

            # ---- attention energies + softmax ---------------------------
            e_ps = ps_e.tile([BL, N], F32, tag="eps")
            first = True
            for half in range(2):
                for c in range(NCHUNK):
                    tin = work.tile([128, BCHUNK * N], BF16, tag="tin")
                    for bb in range(BCHUNK):
                        b = c * BCHUNK + bb
                        nc.vector.tensor_scalar(
                            out=tin[:, bb * N:(bb + 1) * N],
                            in0=r2T[:, half, b, :],
                            scalar1=r1_sb[:, half, b:b + 1],
                            scalar2=None,
                            op0=ADD,
                        )
                    tout = work.tile([128, BCHUNK * N], BF16, tag="tout")
                    nc.scalar.activation(tout, tin, TANH)
                    for bb in range(BCHUNK):
                        b = c * BCHUNK + bb
                        last = (half == 1 and c == NCHUNK - 1 and bb == BCHUNK - 1)
                        nc.tensor.matmul(
                            e_ps,
                            lhsT=vs_s[:, half, b, :],
                            rhs=tout[:, bb * N:(bb + 1) * N],
                            start=first,
                            stop=last,
                        )
                        first = False

            exp_sb = opool.tile([BL, N], BF16, tag="expsb")
            esum = opool.tile([BL, 1], F32, tag="esum")
            nc.scalar.activation(exp_sb, e_ps, EXP, accum_out=esum)
            rsum = opool.tile([BL, 1], F32, tag="rsum")
            nc.vector.reciprocal(rsum, esum)
            alpha = opool.tile([BL, N], BF16, tag="alpha")
            nc.vector.tensor_scalar(out=alpha, in0=exp_sb, scalar1=rsum,
                                    scalar2=None, op0=MULT)
            outv = opool.tile([BL, N], F32, tag="outv")
            nc.vector.tensor_mul(outv, alpha, x_t_sb)
            nc.sync.dma_start(out=out_p[:, t, :], in_=outv)

    nc.compile()
    return nc




def _split_excess_waits(nc: bass.Bass):
    """neuronxcc's walrus only encodes a limited number of sync-wait slots
    per instruction (fewer still for NoOp/ctrl instructions); hoist excess
    waits onto single-wait NoOps in front of the instruction (same engine
    stream, so ordering is preserved)."""
    for bb in nc.main_func.blocks:
        new_insts = []
        for ins in bb.instructions:
            keep_max = 1
            si = ins.sync_info
            if si is not None and si.on_wait and len(si.on_wait) > keep_max:
                waits = list(si.on_wait)
                keep = waits[-keep_max:]
                for w in waits[:-keep_max]:
                    nop = mybir.InstNoOp(
                        name=nc.get_next_instruction_name(),
                        engine=ins.engine,
                        ins=[],
                        outs=[],
                        sync_info=mybir.SyncInfo(on_wait=[w], on_update=[]),
                    )
                    new_insts.append(nop)
                ins.sync_info = mybir.SyncInfo(
                    on_wait=keep, on_update=list(si.on_update or []))
            new_insts.append(ins)
        bb.instructions = new_insts


def _marshal(x, s, h, We, Ue, ve, Wk, Wr, b):
    """Host-side input prep (sharding + weight prepacking, no x-dependent math)."""
    bf = ml_dtypes.bfloat16
    x_bf = x.astype(bf)                                   # [B, T, N]
    xt_bf = np.ascontiguousarray(x_bf.transpose(1, 0, 2)) # [T, B, N]
    hT = np.ascontiguousarray(h.astype(bf).T)             # [M, B]
    sT = np.ascontiguousarray(s.astype(bf).T)

    ue_w = np.ascontiguousarray(Ue.astype(bf).reshape(2, 128, T))
    we_w = np.ascontiguousarray(We.astype(bf).reshape(4, 128, T))
    wc = np.concatenate([Wk, Wr], axis=0)                 # [N+M, 4M]
    wc_w = np.ascontiguousarray(wc.astype(bf).reshape(3, 128, M4))

    vs = np.zeros((128, 2, BL, BL), dtype=bf)
    vef = ve[:, 0].astype(np.float32)
    for half in range(2):
        seg = vef[half * 128:(half + 1) * 128].astype(bf)
        for bb in range(BL):
            vs[:, half, bb, bb] = seg

    # gate order i,f,g,o; i/f/o folded as 0.5*tanh(0.5(z+b))+0.5 -> bias 0.5*b
    coef = np.array([0.5] * 4 + [1.0] * 2 + [0.5] * 2, dtype=np.float32)
    be = (b.astype(np.float32).reshape(8, 128) * coef[:, None]).T  # [128, 8]
    be = np.ascontiguousarray(be)

    ue_blob = ue_w.transpose(1, 0, 2).reshape(128, -1)
    we_blob = we_w.transpose(1, 0, 2).reshape(128, -1)
    wc_blob = wc_w.transpose(1, 0, 2).reshape(128, -1)
    vs_blob = vs.reshape(128, -1)

    in_maps = []
    for i in range(NCORES):
        sl = slice(i * BL, (i + 1) * BL)
        xt_core = xt_bf[:, sl, :].reshape(2, 128, BL, N)
        blob = np.concatenate([
            xt_core.transpose(1, 0, 2, 3).reshape(128, -1),
            ue_blob, we_blob, wc_blob, vs_blob,
        ], axis=1)
        in_maps.append({
            "x_b": np.ascontiguousarray(x_bf[sl]),
            "x_n": np.ascontiguousarray(x_bf[sl].transpose(1, 2, 0)),
            "blob": np.ascontiguousarray(blob),
            "hT0": np.ascontiguousarray(hT[:, sl].reshape(2, 128, BL)),
            "sT0": np.ascontiguousarray(sT[:, sl].reshape(2, 128, BL)),
            "Be_w": be,
        })
    return in_maps


def kernel(**inputs) -> np.ndarray:
    x = np.asarray(inputs["x"])
    s = np.asarray(inputs["s"])
    h = np.asarray(inputs["h"])
    We = np.asarray(inputs["We"])
    Ue = np.asarray(inputs["Ue"])
    ve = np.asarray(inputs["ve"])
    Wk = np.asarray(inputs["Wk"])
    Wr = np.asarray(inputs["Wr"])
    b = np.asarray(inputs["b"])

    in_maps = _marshal(x, s, h, We, Ue, ve, Wk, Wr, b)
    nc = build_nc(T)
    res = run_bass_kernel_spmd(nc, in_maps, core_ids=list(range(NCORES)))
    out = np.concatenate([r["out"] for r in res.results], axis=0)
    return out.astype(np.float32)


if __name__ == "__main__":
    rng = np.random.default_rng(0)
    demo = {
        "x": rng.standard_normal((B, T, N), dtype=np.float32),
        "s": rng.standard_normal((B, M), dtype=np.float32) * 0.1,
        "h": rng.standard_normal((B, M), dtype=np.float32) * 0.1,
        "We": rng.standard_normal((2 * M, T), dtype=np.float32) / np.sqrt(2 * M),
        "Ue": rng.standard_normal((T, T), dtype=np.float32) / np.sqrt(T),
        "ve": rng.standard_normal((T, 1), dtype=np.float32) / np.sqrt(T),
        "Wk": rng.standard_normal((N, M4), dtype=np.float32) / np.sqrt(N),
        "Wr": rng.standard_normal((M, M4), dtype=np.float32) / np.sqrt(M),
        "b": np.zeros((M4,), dtype=np.float32),
    }
    out = kernel(**demo)
    print(out.shape, out.dtype)


# revision 13
# speedup vs baseline: 1.1814x; 1.1814x over previous
"""Trainium2 Bass kernel for the attention-encoder (Bahdanau input attention
+ LSTM cell, T-step recurrence).

Math (per batch row b):
    r2 = einsum('tn,tu->nu', x[b], Ue)                 # [N, T], loop-invariant
    per step t:
        r1 = concat(h, s) @ We                         # [T]
        e[n] = sum_t' ve[t'] * tanh(r1[t'] + r2[n,t']) # [N]
        alpha = softmax_n(e)
        z = x_t @ Wk + h @ Wr + b ; LSTM update (keras gate order i,f,g,o)
        out[b, t, :] = alpha * x[b, t, :]

Strategy: pure data parallelism, batch 512 -> 64 per core on 8 cores.
On-chip layout keeps t' on partitions for the big pass:
    r2T [t'(2x128 part), b, n]  (bf16)
    per step: DVE tensor_scalar adds r1[b,t'] (per-partition scalar),
    ACT does one big tanh per chunk, PE contracts t' against a
    per-b "selector" stationary (col b = ve-half) accumulating
    e into PSUM[b, n] -- natural layout for the free-axis softmax.
LSTM computes z in natural layout ([b, 4M]) with stationaries x_t^T/h^T,
one fused gate tanh (g-gate weights pre-scaled x2 on host so all gates
share scale=0.5), sigmoid-as-tanh to stay in the exp/tanh ACT table set,
then PE-transposes h/s back to the ^T layout the r1/z matmuls need.
"""

import numpy as np
import ml_dtypes
from contextlib import ExitStack

import concourse.bass as bass
import concourse.bacc as bacc
import concourse.tile as tile
from concourse import mybir
from concourse.bass_utils import run_bass_kernel_spmd

B, T, N, M = 512, 256, 128, 256
NCORES = 8
BL = B // NCORES  # 64 batch rows per core
M4 = 4 * M        # 1024

BF16 = mybir.dt.bfloat16
F32 = mybir.dt.float32
TANH = mybir.ActivationFunctionType.Tanh
EXP = mybir.ActivationFunctionType.Exp
ADD = mybir.AluOpType.add
MULT = mybir.AluOpType.mult

BCHUNK = 32             # b-rows per attention chunk (free = BCHUNK*N = 4096)
NCHUNK = BL // BCHUNK   # chunks per t'-half

# blob free-dim offsets (all [128, *] bf16, packed on host by _marshal)
OFF_XT = 0                       # x_tmaj  [p, 2, BL, N]
OFF_UE = OFF_XT + 2 * BL * N     # Ue      [p, 2, T]
OFF_WE = OFF_UE + 2 * T          # We      [p, 4, T]
OFF_WC = OFF_WE + 4 * T          # Wc      [p, 3, M4]  (g cols pre-scaled x2)
OFF_VS = OFF_WC + 3 * M4         # vsel    [p, 2, BL, BL]
BLOB_F = OFF_VS + 2 * BL * BL


def build_nc(t_steps: int = T, with_bias: bool = False) -> bass.Bass:
    nc = bacc.Bacc(None)

    x_p = nc.declare_dram_parameter("x_b", [BL, T, N], BF16, isOutput=False)
    xn_p = nc.declare_dram_parameter("x_n", [T, N, BL], BF16, isOutput=False)
    blob_p = nc.declare_dram_parameter("blob", [128, BLOB_F], BF16, isOutput=False)
    hT_p = nc.declare_dram_parameter("hT0", [2, 128, BL], BF16, isOutput=False)
    sT_p = nc.declare_dram_parameter("sT0", [2, 128, BL], BF16, isOutput=False)
    hn_p = nc.declare_dram_parameter("hn0", [BL, M], BF16, isOutput=False)
    sn_p = nc.declare_dram_parameter("sn0", [BL, M], BF16, isOutput=False)
    id_p = nc.declare_dram_parameter("id64", [BL, BL], BF16, isOutput=False)
    if with_bias:
        bb_p = nc.declare_dram_parameter("biasn", [BL, M4], F32, isOutput=False)
    out_p = nc.declare_dram_parameter("out", [BL, T, N], F32, isOutput=True)

    with tile.TileContext(nc) as tc, ExitStack() as ctx:
        singles = ctx.enter_context(tc.tile_pool(name="singles", bufs=1))

        # ---- resident tensors -------------------------------------------
        blob = singles.tile([128, BLOB_F], BF16)
        r2T = singles.tile([128, 2, BL, N], BF16)      # r2[t', b, n]
        h_bf = singles.tile([128, 2, BL], BF16)        # h^T state
        s_bf = singles.tile([128, 2, BL], BF16)        # s^T state
        h_nat = singles.tile([BL, M], BF16)            # h natural state
        s_nat = singles.tile([BL, M], BF16)            # s natural state
        id_s = singles.tile([BL, BL], BF16)            # 64x64 identity
        if with_bias:
            bb_s = singles.tile([BL, M4], F32)

        x_tmaj = blob[:, OFF_XT:OFF_UE].rearrange(
            "p (h b n) -> p h b n", h=2, b=BL)
        ue_s = blob[:, OFF_UE:OFF_WE].rearrange("p (h t) -> p h t", h=2)
        we_s = blob[:, OFF_WE:OFF_WC].rearrange("p (j t) -> p j t", j=4)
        wc_s = blob[:, OFF_WC:OFF_VS].rearrange("p (j m) -> p j m", j=3)
        vs_s = blob[:, OFF_VS:BLOB_F].rearrange(
            "p (h b m) -> p h b m", h=2, b=BL)

        nc.sync.dma_start(out=blob, in_=blob_p[:])
        nc.sync.dma_start(out=h_bf, in_=hT_p.rearrange("h p b -> p h b"))
        nc.sync.dma_start(out=s_bf, in_=sT_p.rearrange("h p b -> p h b"))
        nc.sync.dma_start(out=h_nat, in_=hn_p[:])
        nc.sync.dma_start(out=s_nat, in_=sn_p[:])
        nc.sync.dma_start(out=id_s, in_=id_p[:])
        if with_bias:
            nc.sync.dma_start(out=bb_s, in_=bb_p[:])

        # ---- precompute r2T: r2[t',b,n] = sum_t Ue[t,t'] x[b,t,n] --------
        with tc.tile_pool(name="pre_ps", bufs=8, space="PSUM") as pre_ps:
            for c in range(2):          # t'-half (output partitions)
                for b in range(BL):
                    r2p = pre_ps.tile([128, N], F32, tag="r2p")
                    for k in range(2):  # contraction half
                        nc.tensor.matmul(
                            r2p,
                            lhsT=ue_s[:, k, c * 128:(c + 1) * 128],
                            rhs=x_tmaj[:, k, b, :],
                            start=(k == 0),
                            stop=(k == 1),
                        )
                    if b % 2 == 0:
                        nc.vector.tensor_copy(r2T[:, c, b, :], r2p)
                    else:
                        nc.scalar.copy(r2T[:, c, b, :], r2p)

        # ---- per-step pools ---------------------------------------------
        work = ctx.enter_context(tc.tile_pool(name="work", bufs=3))
        gate_pool = ctx.enter_context(tc.tile_pool(name="gates", bufs=2))
        ps_z = ctx.enter_context(tc.tile_pool(name="ps_z", bufs=1, space="PSUM"))
        ps_r1 = ctx.enter_context(tc.tile_pool(name="ps_r1", bufs=1, space="PSUM"))
        ps_e = ctx.enter_context(tc.tile_pool(name="ps_e", bufs=2, space="PSUM"))
        ps_tr = ctx.enter_context(tc.tile_pool(name="ps_tr", bufs=1, space="PSUM"))
        xfeed = ctx.enter_context(tc.tile_pool(name="xfeed", bufs=3))
        opool = ctx.enter_context(tc.tile_pool(name="opool", bufs=3))

        def fetch_x(t):
            x_t_sb = xfeed.tile([BL, N], BF16, tag="x_t")
            nc.sync.dma_start(out=x_t_sb, in_=x_p[:, t, :])
            x_tT_sb = xfeed.tile([128, BL], BF16, tag="x_tT")
            nc.sync.dma_start(out=x_tT_sb, in_=xn_p[t])
            return x_t_sb, x_tT_sb

        x_feed = fetch_x(0)

        for t in range(t_steps):
            x_t_sb, x_tT_sb = x_feed
            if t + 1 < t_steps:
                x_feed = fetch_x(t + 1)

            # ---- r1^T = We^T @ [h; s]  -> [t'(2x128), b] ----------------
            r1_ps = ps_r1.tile([128, 2, BL], F32, tag="r1ps")
            for c in range(2):
                for j in range(4):
                    rhs = h_bf[:, j, :] if j < 2 else s_bf[:, j - 2, :]
                    nc.tensor.matmul(
                        r1_ps[:, c, :],
                        lhsT=we_s[:, j, c * 128:(c + 1) * 128],
                        rhs=rhs,
                        start=(j == 0),
                        stop=(j == 3),
                    )
            r1_sb = work.tile([128, 2, BL], F32, tag="r1sb")
            nc.vector.tensor_copy(r1_sb, r1_ps)

            # ---- z natural: [b, 4M] = x_t @ Wk + h @ Wr -----------------
            # stationary = x_tT / hT (k on partitions, cols = b),
            # moving = weight blocks; 6 matmuls of FD=512.
            z_ps = ps_z.tile([BL, M4], F32, tag="zps")
            for mh in range(2):
                sl = slice(mh * 512, (mh + 1) * 512)
                for j in range(3):
                    lhsT = x_tT_sb if j == 0 else h_bf[:, j - 1, :]
                    nc.tensor.matmul(
                        z_ps[:, sl],
                        lhsT=lhsT,
                        rhs=wc_s[:, j, sl],
                        start=(j == 0),
                        stop=(j == 2),
                    )
            if with_bias:
                nc.vector.tensor_add(z_ps, z_ps, bb_s)

            # ---- gates: one fused tanh(0.5 z) over all 4 gates ----------
            t_all = gate_pool.tile([BL, M4], BF16, tag="tall")
            nc.scalar.activation(t_all, z_ps, TANH, scale=0.5)
            t_i = t_all[:, 0:M]
            t_f = t_all[:, M:2 * M]
            t_g = t_all[:, 2 * M:3 * M]   # = tanh(z_g) via host 2x prescale
            t_o = t_all[:, 3 * M:M4]

            fp = gate_pool.tile([BL, M], BF16, tag="fp")
            nc.vector.tensor_scalar(out=fp, in0=t_f, scalar1=0.5, scalar2=0.5,
                                    op0=MULT, op1=ADD)
            v = gate_pool.tile([BL, M], BF16, tag="v")
            nc.vector.tensor_mul(v, fp, s_nat)
            ip = gate_pool.tile([BL, M], BF16, tag="ip")
            nc.vector.tensor_scalar(out=ip, in0=t_i, scalar1=0.5, scalar2=0.5,
                                    op0=MULT, op1=ADD)
            q = gate_pool.tile([BL, M], BF16, tag="q")
            nc.vector.tensor_mul(q, ip, t_g)
            nc.vector.tensor_add(s_nat, v, q)
            tanh_s = gate_pool.tile([BL, M], BF16, tag="tanhs")
            nc.scalar.activation(tanh_s, s_nat, TANH)
            op = gate_pool.tile([BL, M], BF16, tag="op")
            nc.vector.tensor_scalar(out=op, in0=t_o, scalar1=0.5, scalar2=0.5,
                                    op0=MULT, op1=ADD)
            nc.vector.tensor_mul(h_nat, op, tanh_s)

            # ---- transpose new h, s back to ^T layout -------------------
            for c in range(2):
                trh = ps_tr.tile([128, BL], BF16, tag="trh")
                nc.tensor.transpose(trh, h_nat[:, c * 128:(c + 1) * 128], id_s)
                nc.vector.tensor_copy(h_bf[:, c, :], trh)
                trs = ps_tr.tile([128, BL], BF16, tag="trs")
                nc.tensor.transpose(trs, s_nat[:, c * 128:(c + 1) * 128], id_s)
                nc.vector.tensor_copy(s_bf[:, c, :], trs)

            # ---- attention energies + softmax ---------------------------
            e_ps = ps_e.tile([BL, N], F32, tag="eps")
            first = True
            for half in range(2):
                for c in range(NCHUNK):
                    tin = work.tile([128, BCHUNK * N], BF16, tag="tin")
                    for bb in range(BCHUNK):
                        b = c * BCHUNK + bb
                        nc.vector.tensor_scalar(
                            out=tin[:, bb * N:(bb + 1) * N],
                            in0=r2T[:, half, b, :],
                            scalar1=r1_sb[:, half, b:b + 1],
                            scalar2=None,
                            op0=ADD,
                        )
                    tout = work.tile([128, BCHUNK * N], BF16, tag="tout")
                    nc.scalar.activation(tout, tin, TANH)
                    for bb in range(BCHUNK):
                        b = c * BCHUNK + bb
                        last = (half == 1 and c == NCHUNK - 1 and bb == BCHUNK - 1)
                        nc.tensor.matmul(
                            e_ps,
                            lhsT=vs_s[:, half, b, :],
                            rhs=tout[:, bb * N:(bb + 1) * N],
                            start=first,
                            stop=last,
                        )
                        first = False

            exp_sb = opool.tile([BL, N], BF16, tag="expsb")
            esum = opool.tile([BL, 1], F32, tag="esum")
            nc.scalar.activation(exp_sb, e_ps, EXP, accum_out=esum)
            rsum = opool.tile([BL, 1], F32, tag="rsum")
            nc.vector.reciprocal(rsum, esum)
            alpha = opool.tile([BL, N], BF16, tag="alpha")
            nc.vector.tensor_scalar(out=alpha, in0=exp_sb, scalar1=rsum,
                                    scalar2=None, op0=MULT)
            outv = opool.tile([BL, N], F32, tag="outv")
            nc.vector.tensor_mul(outv, alpha, x_t_sb)
            nc.sync.dma_start(out=out_p[:, t, :], in_=outv)

    nc.compile()
    return nc


def _marshal(x, s, h, We, Ue, ve, Wk, Wr, b):
    """Host-side input prep (sharding + weight prepacking, no x-dependent math)."""
    bf = ml_dtypes.bfloat16
    x_bf = x.astype(bf)                                   # [B, T, N]
    xt_bf = np.ascontiguousarray(x_bf.transpose(1, 0, 2)) # [T, B, N]
    hT = np.ascontiguousarray(h.astype(bf).T)             # [M, B]
    sT = np.ascontiguousarray(s.astype(bf).T)

    ue_w = np.ascontiguousarray(Ue.astype(bf).reshape(2, 128, T))
    we_w = np.ascontiguousarray(We.astype(bf).reshape(4, 128, T))
    wc = np.concatenate([Wk, Wr], axis=0).astype(np.float32)  # [N+M, 4M]
    wc[:, 2 * M:3 * M] *= 2.0    # pre-scale g gate so tanh uses scale=0.5
    wc_w = np.ascontiguousarray(wc.astype(bf).reshape(3, 128, M4))

    vs = np.zeros((128, 2, BL, BL), dtype=bf)
    vef = ve[:, 0].astype(np.float32)
    for half in range(2):
        seg = vef[half * 128:(half + 1) * 128].astype(bf)
        for bb in range(BL):
            vs[:, half, bb, bb] = seg

    ue_blob = ue_w.transpose(1, 0, 2).reshape(128, -1)
    we_blob = we_w.transpose(1, 0, 2).reshape(128, -1)
    wc_blob = wc_w.transpose(1, 0, 2).reshape(128, -1)
    vs_blob = vs.reshape(128, -1)
    id64 = np.eye(BL, dtype=bf)

    with_bias = bool(np.any(b))
    bias2 = b.astype(np.float32).copy()
    bias2[2 * M:3 * M] *= 2.0
    bias_nat = np.ascontiguousarray(
        np.broadcast_to(bias2, (BL, M4)).astype(np.float32))

    in_maps = []
    for i in range(NCORES):
        sl = slice(i * BL, (i + 1) * BL)
        xt_core = xt_bf[:, sl, :].reshape(2, 128, BL, N)
        blob = np.concatenate([
            xt_core.transpose(1, 0, 2, 3).reshape(128, -1),
            ue_blob, we_blob, wc_blob, vs_blob,
        ], axis=1)
        m = {
            "x_b": np.ascontiguousarray(x_bf[sl]),
            "x_n": np.ascontiguousarray(x_bf[sl].transpose(1, 2, 0)),
            "blob": np.ascontiguousarray(blob),
            "hT0": np.ascontiguousarray(hT[:, sl].reshape(2, 128, BL)),
            "sT0": np.ascontiguousarray(sT[:, sl].reshape(2, 128, BL)),
            "hn0": np.ascontiguousarray(h[sl].astype(bf)),
            "sn0": np.ascontiguousarray(s[sl].astype(bf)),
            "id64": id64,
        }
        if with_bias:
            m["biasn"] = bias_nat
        in_maps.append(m)
    return in_maps, with_bias


def kernel(**inputs) -> np.ndarray:
    x = np.asarray(inputs["x"])
    s = np.asarray(inputs["s"])
    h = np.asarray(inputs["h"])
    We = np.asarray(inputs["We"])
    Ue = np.asarray(inputs["Ue"])
    ve = np.asarray(inputs["ve"])
    Wk = np.asarray(inputs["Wk"])
    Wr = np.asarray(inputs["Wr"])
    b = np.asarray(inputs["b"])

    in_maps, with_bias = _marshal(x, s, h, We, Ue, ve, Wk, Wr, b)
    nc = build_nc(T, with_bias=with_bias)
    res = run_bass_kernel_spmd(nc, in_maps, core_ids=list(range(NCORES)))
    out = np.concatenate([r["out"] for r in res.results], axis=0)
    return out.astype(np.float32)


if __name__ == "__main__":
    rng = np.random.default_rng(0)
    demo = {
        "x": rng.standard_normal((B, T, N), dtype=np.float32),
        "s": rng.standard_normal((B, M), dtype=np.float32) * 0.1,
        "h": rng.standard_normal((B, M), dtype=np.float32) * 0.1,
        "We": rng.standard_normal((2 * M, T), dtype=np.float32) / np.sqrt(2 * M),
        "Ue": rng.standard_normal((T, T), dtype=np.float32) / np.sqrt(T),
        "ve": rng.standard_normal((T, 1), dtype=np.float32) / np.sqrt(T),
        "Wk": rng.standard_normal((N, M4), dtype=np.float32) / np.sqrt(N),
        "Wr": rng.standard_normal((M, M4), dtype=np.float32) / np.sqrt(M),
        "b": np.zeros((M4,), dtype=np.float32),
    }
    out = kernel(**demo)
    print(out.shape, out.dtype)


# revision 14
# speedup vs baseline: 6.4599x; 5.4682x over previous
"""Trainium2 Bass kernel for the attention-encoder (Bahdanau input attention
+ LSTM cell, T-step recurrence).

Math (per batch row b):
    r2 = einsum('tn,tu->nu', x[b], Ue)                 # [N, T], loop-invariant
    per step t:
        r1 = concat(h, s) @ We                         # [T]
        e[n] = sum_t' ve[t'] * tanh(r1[t'] + r2[n,t']) # [N]
        alpha = softmax_n(e)
        z = x_t @ Wk + h @ Wr + b ; LSTM update (keras gate order i,f,g,o)
        out[b, t, :] = alpha * x[b, t, :]

Strategy: pure data parallelism, batch 512 -> 64 per core on 8 cores.
On-chip layout keeps t' on partitions for the big pass:
    r2T [t'(2x128 part), b, n]  (bf16)
    per step: DVE tensor_scalar adds r1[b,t'] (per-partition scalar),
    ACT does one big tanh per chunk, PE contracts t' against a
    per-b "selector" stationary (col b = ve-half) accumulating
    e into PSUM[b, n] -- natural layout for the free-axis softmax.
LSTM computes z in natural layout ([b, 4M]) with stationaries x_t^T/h^T,
one fused gate tanh (g-gate weights pre-scaled x2 on host so all gates
share scale=0.5), sigmoid-as-tanh to stay in the exp/tanh ACT table set,
then PE-transposes h/s back to the ^T layout the r1/z matmuls need.
"""

import numpy as np
import ml_dtypes
from contextlib import ExitStack

import concourse.bass as bass
import concourse.bacc as bacc
import concourse.tile as tile
from concourse import mybir
from concourse.bass_utils import run_bass_kernel_spmd

B, T, N, M = 512, 256, 128, 256
NCORES = 8
BL = B // NCORES  # 64 batch rows per core
M4 = 4 * M        # 1024

BF16 = mybir.dt.bfloat16
F32 = mybir.dt.float32
TANH = mybir.ActivationFunctionType.Tanh
EXP = mybir.ActivationFunctionType.Exp
ADD = mybir.AluOpType.add
MULT = mybir.AluOpType.mult

BCHUNK = 32             # b-rows per attention chunk (free = BCHUNK*N = 4096)
NCHUNK = BL // BCHUNK   # chunks per t'-half

# blob free-dim offsets (all [128, *] bf16, packed on host by _marshal)
OFF_XT = 0                       # x_tmaj  [p, 2, BL, N]
OFF_UE = OFF_XT + 2 * BL * N     # Ue      [p, 2, T]
OFF_WE = OFF_UE + 2 * T          # We      [p, 4, T]
OFF_WC = OFF_WE + 4 * T          # Wc      [p, 3, M4]  (g cols pre-scaled x2)
OFF_VS = OFF_WC + 3 * M4         # vsel    [p, 2, BL, BL]
BLOB_F = OFF_VS + 2 * BL * BL


def build_nc(t_steps: int = T, with_bias: bool = False,
             repeats: int = 1) -> bass.Bass:
    nc = bacc.Bacc(None)

    x_p = nc.declare_dram_parameter("x_b", [BL, T, N], BF16, isOutput=False)
    xn_p = nc.declare_dram_parameter("x_n", [T, N, BL], BF16, isOutput=False)
    blob_p = nc.declare_dram_parameter("blob", [128, BLOB_F], BF16, isOutput=False)
    hT_p = nc.declare_dram_parameter("hT0", [2, 128, BL], BF16, isOutput=False)
    sT_p = nc.declare_dram_parameter("sT0", [2, 128, BL], BF16, isOutput=False)
    hn_p = nc.declare_dram_parameter("hn0", [BL, M], BF16, isOutput=False)
    sn_p = nc.declare_dram_parameter("sn0", [BL, M], BF16, isOutput=False)
    id_p = nc.declare_dram_parameter("id64", [BL, BL], BF16, isOutput=False)
    if with_bias:
        bb_p = nc.declare_dram_parameter("biasn", [BL, M4], F32, isOutput=False)
    out_p = nc.declare_dram_parameter("out", [BL, T, N], F32, isOutput=True)

    with tile.TileContext(nc) as tc, ExitStack() as ctx:
        singles = ctx.enter_context(tc.tile_pool(name="singles", bufs=1))

        # ---- resident tensors -------------------------------------------
        blob = singles.tile([128, BLOB_F], BF16)
        r2T = singles.tile([128, 2, BL, N], BF16)      # r2[t', b, n]
        h_bf = singles.tile([128, 2, BL], BF16)        # h^T state
        s_bf = singles.tile([128, 2, BL], BF16)        # s^T state
        h_nat = singles.tile([BL, M], BF16)            # h natural state
        s_nat = singles.tile([BL, M], BF16)            # s natural state
        id_s = singles.tile([BL, BL], BF16)            # 64x64 identity
        if with_bias:
            bb_s = singles.tile([BL, M4], F32)

        x_tmaj = blob[:, OFF_XT:OFF_UE].rearrange(
            "p (h b n) -> p h b n", h=2, b=BL)
        ue_s = blob[:, OFF_UE:OFF_WE].rearrange("p (h t) -> p h t", h=2)
        we_s = blob[:, OFF_WE:OFF_WC].rearrange("p (j t) -> p j t", j=4)
        wc_s = blob[:, OFF_WC:OFF_VS].rearrange("p (j m) -> p j m", j=3)
        vs_s = blob[:, OFF_VS:BLOB_F].rearrange(
            "p (h b m) -> p h b m", h=2, b=BL)

        nc.sync.dma_start(out=blob, in_=blob_p[:])
        nc.sync.dma_start(out=h_bf, in_=hT_p.rearrange("h p b -> p h b"))
        nc.sync.dma_start(out=s_bf, in_=sT_p.rearrange("h p b -> p h b"))
        nc.sync.dma_start(out=h_nat, in_=hn_p[:])
        nc.sync.dma_start(out=s_nat, in_=sn_p[:])
        nc.sync.dma_start(out=id_s, in_=id_p[:])
        if with_bias:
            nc.sync.dma_start(out=bb_s, in_=bb_p[:])

        # ---- precompute r2T: r2[t',b,n] = sum_t Ue[t,t'] x[b,t,n] --------
        with tc.tile_pool(name="pre_ps", bufs=8, space="PSUM") as pre_ps:
            for c in range(2):          # t'-half (output partitions)
                for b in range(BL):
                    r2p = pre_ps.tile([128, N], F32, tag="r2p")
                    for k in range(2):  # contraction half
                        nc.tensor.matmul(
                            r2p,
                            lhsT=ue_s[:, k, c * 128:(c + 1) * 128],
                            rhs=x_tmaj[:, k, b, :],
                            start=(k == 0),
                            stop=(k == 1),
                        )
                    if b % 2 == 0:
                        nc.vector.tensor_copy(r2T[:, c, b, :], r2p)
                    else:
                        nc.scalar.copy(r2T[:, c, b, :], r2p)

        # ---- per-step pools ---------------------------------------------
        work = ctx.enter_context(tc.tile_pool(name="work", bufs=3))
        gate_pool = ctx.enter_context(tc.tile_pool(name="gates", bufs=2))
        ps_z = ctx.enter_context(tc.tile_pool(name="ps_z", bufs=1, space="PSUM"))
        ps_r1 = ctx.enter_context(tc.tile_pool(name="ps_r1", bufs=1, space="PSUM"))
        ps_e = ctx.enter_context(tc.tile_pool(name="ps_e", bufs=2, space="PSUM"))
        ps_tr = ctx.enter_context(tc.tile_pool(name="ps_tr", bufs=1, space="PSUM"))
        xfeed = ctx.enter_context(tc.tile_pool(name="xfeed", bufs=3))
        opool = ctx.enter_context(tc.tile_pool(name="opool", bufs=3))

        def fetch_x(t):
            x_t_sb = xfeed.tile([BL, N], BF16, tag="x_t")
            nc.sync.dma_start(out=x_t_sb, in_=x_p[:, t, :])
            x_tT_sb = xfeed.tile([128, BL], BF16, tag="x_tT")
            nc.sync.dma_start(out=x_tT_sb, in_=xn_p[t])
            return x_t_sb, x_tT_sb

        x_feed = fetch_x(0)

        for t in [tt for _ in range(repeats) for tt in range(t_steps)]:
            x_t_sb, x_tT_sb = x_feed
            if t + 1 < t_steps:
                x_feed = fetch_x(t + 1)

            # ---- r1^T = We^T @ [h; s]  -> [t'(2x128), b] ----------------
            r1_ps = ps_r1.tile([128, 2, BL], F32, tag="r1ps")
            for c in range(2):
                for j in range(4):
                    rhs = h_bf[:, j, :] if j < 2 else s_bf[:, j - 2, :]
                    nc.tensor.matmul(
                        r1_ps[:, c, :],
                        lhsT=we_s[:, j, c * 128:(c + 1) * 128],
                        rhs=rhs,
                        start=(j == 0),
                        stop=(j == 3),
                    )
            r1_sb = work.tile([128, 2, BL], F32, tag="r1sb")
            nc.vector.tensor_copy(r1_sb, r1_ps)

            # ---- z natural: [b, 4M] = x_t @ Wk + h @ Wr -----------------
            # stationary = x_tT / hT (k on partitions, cols = b),
            # moving = weight blocks; 6 matmuls of FD=512.
            z_ps = ps_z.tile([BL, M4], F32, tag="zps")
            for mh in range(2):
                sl = slice(mh * 512, (mh + 1) * 512)
                for j in range(3):
                    lhsT = x_tT_sb if j == 0 else h_bf[:, j - 1, :]
                    nc.tensor.matmul(
                        z_ps[:, sl],
                        lhsT=lhsT,
                        rhs=wc_s[:, j, sl],
                        start=(j == 0),
                        stop=(j == 2),
                    )
            if with_bias:
                nc.vector.tensor_add(z_ps, z_ps, bb_s)

            # ---- gates: one fused tanh(0.5 z) over all 4 gates ----------
            t_all = gate_pool.tile([BL, M4], BF16, tag="tall")
            nc.scalar.activation(t_all, z_ps, TANH, scale=0.5)
            t_i = t_all[:, 0:M]
            t_f = t_all[:, M:2 * M]
            t_g = t_all[:, 2 * M:3 * M]   # = tanh(z_g) via host 2x prescale
            t_o = t_all[:, 3 * M:M4]

            fp = gate_pool.tile([BL, M], BF16, tag="fp")
            nc.vector.tensor_scalar(out=fp, in0=t_f, scalar1=0.5, scalar2=0.5,
                                    op0=MULT, op1=ADD)
            v = gate_pool.tile([BL, M], BF16, tag="v")
            nc.vector.tensor_mul(v, fp, s_nat)
            ip = gate_pool.tile([BL, M], BF16, tag="ip")
            nc.vector.tensor_scalar(out=ip, in0=t_i, scalar1=0.5, scalar2=0.5,
                                    op0=MULT, op1=ADD)
            q = gate_pool.tile([BL, M], BF16, tag="q")
            nc.vector.tensor_mul(q, ip, t_g)
            nc.vector.tensor_add(s_nat, v, q)
            tanh_s = gate_pool.tile([BL, M], BF16, tag="tanhs")
            nc.scalar.activation(tanh_s, s_nat, TANH)
            op = gate_pool.tile([BL, M], BF16, tag="op")
            nc.vector.tensor_scalar(out=op, in0=t_o, scalar1=0.5, scalar2=0.5,
                                    op0=MULT, op1=ADD)
            nc.vector.tensor_mul(h_nat, op, tanh_s)

            # ---- transpose new h, s back to ^T layout -------------------
            for c in range(2):
                trh = ps_tr.tile([128, BL], BF16, tag="trh")
                nc.tensor.transpose(trh, h_nat[:, c * 128:(c + 1) * 128], id_s)
                nc.vector.tensor_copy(h_bf[:, c, :], trh)
                trs = ps_tr.tile([128, BL], BF16, tag="trs")
                nc.tensor.transpose(trs, s_nat[:, c * 128:(c + 1) * 128], id_s)
                nc.vector.tensor_copy(s_bf[:, c, :], trs)

            # ---- attention energies + softmax ---------------------------
            e_ps = ps_e.tile([BL, N], F32, tag="eps")
            first = True
            for half in range(2):
                for c in range(NCHUNK):
                    tin = work.tile([128, BCHUNK * N], BF16, tag="tin")
                    for bb in range(BCHUNK):
                        b = c * BCHUNK + bb
                        nc.vector.tensor_scalar(
                            out=tin[:, bb * N:(bb + 1) * N],
                            in0=r2T[:, half, b, :],
                            scalar1=r1_sb[:, half, b:b + 1],
                            scalar2=None,
                            op0=ADD,
                        )
                    tout = work.tile([128, BCHUNK * N], BF16, tag="tout")
                    nc.scalar.activation(tout, tin, TANH)
                    for bb in range(BCHUNK):
                        b = c * BCHUNK + bb
                        last = (half == 1 and c == NCHUNK - 1 and bb == BCHUNK - 1)
                        nc.tensor.matmul(
                            e_ps,
                            lhsT=vs_s[:, half, b, :],
                            rhs=tout[:, bb * N:(bb + 1) * N],
                            start=first,
                            stop=last,
                        )
                        first = False

            exp_sb = opool.tile([BL, N], BF16, tag="expsb")
            esum = opool.tile([BL, 1], F32, tag="esum")
            nc.scalar.activation(exp_sb, e_ps, EXP, accum_out=esum)
            rsum = opool.tile([BL, 1], F32, tag="rsum")
            nc.vector.reciprocal(rsum, esum)
            alpha = opool.tile([BL, N], BF16, tag="alpha")
            nc.vector.tensor_scalar(out=alpha, in0=exp_sb, scalar1=rsum,
                                    scalar2=None, op0=MULT)
            outv = opool.tile([BL, N], F32, tag="outv")
            nc.vector.tensor_mul(outv, alpha, x_t_sb)
            nc.sync.dma_start(out=out_p[:, t, :], in_=outv)

    nc.compile()
    return nc


def _marshal(x, s, h, We, Ue, ve, Wk, Wr, b):
    """Host-side input prep (sharding + weight prepacking, no x-dependent math)."""
    bf = ml_dtypes.bfloat16
    x_bf = x.astype(bf)                                   # [B, T, N]
    xt_bf = np.ascontiguousarray(x_bf.transpose(1, 0, 2)) # [T, B, N]
    hT = np.ascontiguousarray(h.astype(bf).T)             # [M, B]
    sT = np.ascontiguousarray(s.astype(bf).T)

    ue_w = np.ascontiguousarray(Ue.astype(bf).reshape(2, 128, T))
    we_w = np.ascontiguousarray(We.astype(bf).reshape(4, 128, T))
    wc = np.concatenate([Wk, Wr], axis=0).astype(np.float32)  # [N+M, 4M]
    wc[:, 2 * M:3 * M] *= 2.0    # pre-scale g gate so tanh uses scale=0.5
    wc_w = np.ascontiguousarray(wc.astype(bf).reshape(3, 128, M4))

    vs = np.zeros((128, 2, BL, BL), dtype=bf)
    vef = ve[:, 0].astype(np.float32)
    for half in range(2):
        seg = vef[half * 128:(half + 1) * 128].astype(bf)
        for bb in range(BL):
            vs[:, half, bb, bb] = seg

    ue_blob = ue_w.transpose(1, 0, 2).reshape(128, -1)
    we_blob = we_w.transpose(1, 0, 2).reshape(128, -1)
    wc_blob = wc_w.transpose(1, 0, 2).reshape(128, -1)
    vs_blob = vs.reshape(128, -1)
    id64 = np.eye(BL, dtype=bf)

    with_bias = bool(np.any(b))
    bias2 = b.astype(np.float32).copy()
    bias2[2 * M:3 * M] *= 2.0
    bias_nat = np.ascontiguousarray(
        np.broadcast_to(bias2, (BL, M4)).astype(np.float32))

    in_maps = []
    for i in range(NCORES):
        sl = slice(i * BL, (i + 1) * BL)
        xt_core = xt_bf[:, sl, :].reshape(2, 128, BL, N)
        blob = np.concatenate([
            xt_core.transpose(1, 0, 2, 3).reshape(128, -1),
            ue_blob, we_blob, wc_blob, vs_blob,
        ], axis=1)
        m = {
            "x_b": np.ascontiguousarray(x_bf[sl]),
            "x_n": np.ascontiguousarray(x_bf[sl].transpose(1, 2, 0)),
            "blob": np.ascontiguousarray(blob),
            "hT0": np.ascontiguousarray(hT[:, sl].reshape(2, 128, BL)),
            "sT0": np.ascontiguousarray(sT[:, sl].reshape(2, 128, BL)),
            "hn0": np.ascontiguousarray(h[sl].astype(bf)),
            "sn0": np.ascontiguousarray(s[sl].astype(bf)),
            "id64": id64,
        }
        if with_bias:
            m["biasn"] = bias_nat
        in_maps.append(m)
    return in_maps, with_bias


def kernel(**inputs) -> np.ndarray:
    x = np.asarray(inputs["x"])
    s = np.asarray(inputs["s"])
    h = np.asarray(inputs["h"])
    We = np.asarray(inputs["We"])
    Ue = np.asarray(inputs["Ue"])
    ve = np.asarray(inputs["ve"])
    Wk = np.asarray(inputs["Wk"])
    Wr = np.asarray(inputs["Wr"])
    b = np.asarray(inputs["b"])

    in_maps, with_bias = _marshal(x, s, h, We, Ue, ve, Wk, Wr, b)
    nc = build_nc(T, with_bias=with_bias)
    res = run_bass_kernel_spmd(nc, in_maps, core_ids=list(range(NCORES)))
    out = np.concatenate([r["out"] for r in res.results], axis=0)
    return out.astype(np.float32)


if __name__ == "__main__":
    rng = np.random.default_rng(0)
    demo = {
        "x": rng.standard_normal((B, T, N), dtype=np.float32),
        "s": rng.standard_normal((B, M), dtype=np.float32) * 0.1,
        "h": rng.standard_normal((B, M), dtype=np.float32) * 0.1,
        "We": rng.standard_normal((2 * M, T), dtype=np.float32) / np.sqrt(2 * M),
        "Ue": rng.standard_normal((T, T), dtype=np.float32) / np.sqrt(T),
        "ve": rng.standard_normal((T, 1), dtype=np.float32) / np.sqrt(T),
        "Wk": rng.standard_normal((N, M4), dtype=np.float32) / np.sqrt(N),
        "Wr": rng.standard_normal((M, M4), dtype=np.float32) / np.sqrt(M),
        "b": np.zeros((M4,), dtype=np.float32),
    }
    out = kernel(**demo)
    print(out.shape, out.dtype)


# revision 17
# speedup vs baseline: 6.5462x; 1.0134x over previous
"""Trainium2 Bass kernel for the attention-encoder (Bahdanau input attention
+ LSTM cell, T-step recurrence).

Math (per batch row b):
    r2 = einsum('tn,tu->nu', x[b], Ue)                 # [N, T], loop-invariant
    per step t:
        r1 = concat(h, s) @ We                         # [T]
        e[n] = sum_t' ve[t'] * tanh(r1[t'] + r2[n,t']) # [N]
        alpha = softmax_n(e)
        z = x_t @ Wk + h @ Wr + b ; LSTM update (keras gate order i,f,g,o)
        out[b, t, :] = alpha * x[b, t, :]

Strategy: pure data parallelism, batch 512 -> 64 per core on 8 cores.
On-chip layout keeps t' on partitions for the big pass:
    r2T [t'(2x128 part), b, n]  (bf16)
    per step: DVE tensor_scalar adds r1[b,t'] (per-partition scalar),
    ACT does one big tanh per chunk, PE contracts t' against a
    per-b "selector" stationary (col b = ve-half) accumulating
    e into PSUM[b, n] -- natural layout for the free-axis softmax.
LSTM computes z in natural layout ([b, 4M]) with stationaries x_t^T/h^T,
one fused gate tanh (g-gate weights pre-scaled x2 on host so all gates
share scale=0.5), sigmoid-as-tanh to stay in the exp/tanh ACT table set,
then PE-transposes h/s back to the ^T layout the r1/z matmuls need.
"""

import numpy as np
import ml_dtypes
from contextlib import ExitStack

import concourse.bass as bass
import concourse.bacc as bacc
import concourse.tile as tile
from concourse import mybir
from concourse.bass_utils import run_bass_kernel_spmd

B, T, N, M = 512, 256, 128, 256
NCORES = 8
BL = B // NCORES  # 64 batch rows per core
M4 = 4 * M        # 1024

BF16 = mybir.dt.bfloat16
F32 = mybir.dt.float32
TANH = mybir.ActivationFunctionType.Tanh
EXP = mybir.ActivationFunctionType.Exp
ADD = mybir.AluOpType.add
MULT = mybir.AluOpType.mult

BCHUNK = 32             # b-rows per attention chunk (free = BCHUNK*N = 4096)
NCHUNK = BL // BCHUNK   # chunks per t'-half

# blob free-dim offsets (all [128, *] bf16, packed on host by _marshal)
OFF_XT = 0                       # x_tmaj  [p, 2, BL, N]
OFF_UE = OFF_XT + 2 * BL * N     # Ue      [p, 2, T]
OFF_WE = OFF_UE + 2 * T          # We      [p, 4, T]
OFF_WC = OFF_WE + 4 * T          # Wc      [p, 3, M4]  (g cols pre-scaled x2)
OFF_VS = OFF_WC + 3 * M4         # vsel    [p, 2, BL, BL]
BLOB_F = OFF_VS + 2 * BL * BL


def build_nc(t_steps: int = T, with_bias: bool = False,
             repeats: int = 1) -> bass.Bass:
    nc = bacc.Bacc(None)

    x_p = nc.declare_dram_parameter("x_b", [BL, T, N], BF16, isOutput=False)
    xn_p = nc.declare_dram_parameter("x_n", [T, N, BL], BF16, isOutput=False)
    blob_p = nc.declare_dram_parameter("blob", [128, BLOB_F], BF16, isOutput=False)
    hT_p = nc.declare_dram_parameter("hT0", [2, 128, BL], BF16, isOutput=False)
    sT_p = nc.declare_dram_parameter("sT0", [2, 128, BL], BF16, isOutput=False)
    hn_p = nc.declare_dram_parameter("hn0", [BL, M], BF16, isOutput=False)
    sn_p = nc.declare_dram_parameter("sn0", [BL, M], BF16, isOutput=False)
    id_p = nc.declare_dram_parameter("id64", [BL, BL], BF16, isOutput=False)
    if with_bias:
        bb_p = nc.declare_dram_parameter("biasn", [BL, M4], F32, isOutput=False)
    out_p = nc.declare_dram_parameter("out", [BL, T, N], F32, isOutput=True)

    with tile.TileContext(nc) as tc, ExitStack() as ctx:
        singles = ctx.enter_context(tc.tile_pool(name="singles", bufs=1))

        # ---- resident tensors -------------------------------------------
        blob = singles.tile([128, BLOB_F], BF16)
        r2T = singles.tile([128, 2, BL, N], BF16)      # r2[t', b, n]
        h_bf = singles.tile([128, 2, BL], BF16)        # h^T state
        s_bf = singles.tile([128, 2, BL], BF16)        # s^T state
        h_nat = singles.tile([BL, M], BF16)            # h natural state
        s_nat = singles.tile([BL, M], BF16)            # s natural state
        id_s = singles.tile([BL, BL], BF16)            # 64x64 identity
        if with_bias:
            bb_s = singles.tile([BL, M4], F32)

        x_tmaj = blob[:, OFF_XT:OFF_UE].rearrange(
            "p (h b n) -> p h b n", h=2, b=BL)
        ue_s = blob[:, OFF_UE:OFF_WE].rearrange("p (h t) -> p h t", h=2)
        we_s = blob[:, OFF_WE:OFF_WC].rearrange("p (j t) -> p j t", j=4)
        wc_s = blob[:, OFF_WC:OFF_VS].rearrange("p (j m) -> p j m", j=3)
        vs_s = blob[:, OFF_VS:BLOB_F].rearrange(
            "p (h b m) -> p h b m", h=2, b=BL)

        nc.sync.dma_start(out=blob, in_=blob_p[:])
        nc.sync.dma_start(out=h_bf, in_=hT_p.rearrange("h p b -> p h b"))
        nc.sync.dma_start(out=s_bf, in_=sT_p.rearrange("h p b -> p h b"))
        nc.sync.dma_start(out=h_nat, in_=hn_p[:])
        nc.sync.dma_start(out=s_nat, in_=sn_p[:])
        nc.sync.dma_start(out=id_s, in_=id_p[:])
        if with_bias:
            nc.sync.dma_start(out=bb_s, in_=bb_p[:])

        # ---- precompute r2T: r2[t',b,n] = sum_t Ue[t,t'] x[b,t,n] --------
        with tc.tile_pool(name="pre_ps", bufs=8, space="PSUM") as pre_ps:
            for c in range(2):          # t'-half (output partitions)
                for b in range(BL):
                    r2p = pre_ps.tile([128, N], F32, tag="r2p")
                    for k in range(2):  # contraction half
                        nc.tensor.matmul(
                            r2p,
                            lhsT=ue_s[:, k, c * 128:(c + 1) * 128],
                            rhs=x_tmaj[:, k, b, :],
                            start=(k == 0),
                            stop=(k == 1),
                        )
                    if b % 2 == 0:
                        nc.vector.tensor_copy(r2T[:, c, b, :], r2p)
                    else:
                        nc.scalar.copy(r2T[:, c, b, :], r2p)

        # ---- per-step pools ---------------------------------------------
        work = ctx.enter_context(tc.tile_pool(name="work", bufs=3))
        gate_pool = ctx.enter_context(tc.tile_pool(name="gates", bufs=2))
        ps_z = ctx.enter_context(tc.tile_pool(name="ps_z", bufs=1, space="PSUM"))
        ps_r1 = ctx.enter_context(tc.tile_pool(name="ps_r1", bufs=1, space="PSUM"))
        ps_e = ctx.enter_context(tc.tile_pool(name="ps_e", bufs=2, space="PSUM"))
        ps_tr = ctx.enter_context(tc.tile_pool(name="ps_tr", bufs=1, space="PSUM"))
        xfeed = ctx.enter_context(tc.tile_pool(name="xfeed", bufs=3))
        opool = ctx.enter_context(tc.tile_pool(name="opool", bufs=3))

        def fetch_x(t):
            x_t_sb = xfeed.tile([BL, N], BF16, tag="x_t")
            nc.sync.dma_start(out=x_t_sb, in_=x_p[:, t, :])
            x_tT_sb = xfeed.tile([128, BL], BF16, tag="x_tT")
            nc.sync.dma_start(out=x_tT_sb, in_=xn_p[t])
            return x_t_sb, x_tT_sb

        x_feed = fetch_x(0)

        for t in [tt for _ in range(repeats) for tt in range(t_steps)]:
            x_t_sb, x_tT_sb = x_feed
            if t + 1 < t_steps:
                x_feed = fetch_x(t + 1)

            # ---- r1^T = We^T @ [h; s]  -> [t'(2x128), b] ----------------
            r1_ps = ps_r1.tile([128, 2, BL], F32, tag="r1ps")
            for c in range(2):
                for j in range(4):
                    rhs = h_bf[:, j, :] if j < 2 else s_bf[:, j - 2, :]
                    nc.tensor.matmul(
                        r1_ps[:, c, :],
                        lhsT=we_s[:, j, c * 128:(c + 1) * 128],
                        rhs=rhs,
                        start=(j == 0),
                        stop=(j == 3),
                    )
            r1_sb = work.tile([128, 2, BL], F32, tag="r1sb")
            nc.vector.tensor_copy(r1_sb, r1_ps)

            # ---- z natural: [b, 4M] = x_t @ Wk + h @ Wr -----------------
            # stationary = x_tT / hT (k on partitions, cols = b),
            # moving = weight blocks; 6 matmuls of FD=512.
            z_ps = ps_z.tile([BL, M4], F32, tag="zps")
            for mh in range(2):
                sl = slice(mh * 512, (mh + 1) * 512)
                for j in range(3):
                    lhsT = x_tT_sb if j == 0 else h_bf[:, j - 1, :]
                    nc.tensor.matmul(
                        z_ps[:, sl],
                        lhsT=lhsT,
                        rhs=wc_s[:, j, sl],
                        start=(j == 0),
                        stop=(j == 2),
                    )
            if with_bias:
                nc.vector.tensor_add(z_ps, z_ps, bb_s)

            # ---- gates: one fused tanh(0.5 z) over all 4 gates ----------
            t_all = gate_pool.tile([BL, M4], BF16, tag="tall")
            nc.scalar.activation(t_all, z_ps, TANH, scale=0.5)
            t_i = t_all[:, 0:M]
            t_f = t_all[:, M:2 * M]
            t_g = t_all[:, 2 * M:3 * M]   # = tanh(z_g) via host 2x prescale
            t_o = t_all[:, 3 * M:M4]

            # states are doubled (H=2h, S=2s; the 0.5 is folded into the
            # We/Wr weight rows on the host):
            #   S_new = 0.5*(t_f+1)*S + (t_i+1)*t_g
            #   H_new = (t_o+1)*tanh(0.5*S_new)
            v = gate_pool.tile([BL, M], BF16, tag="v")
            nc.vector.scalar_tensor_tensor(v, t_f, 1.0, s_nat, ADD, MULT)
            q = gate_pool.tile([BL, M], BF16, tag="q")
            nc.vector.scalar_tensor_tensor(q, t_i, 1.0, t_g, ADD, MULT)
            nc.vector.scalar_tensor_tensor(s_nat, v, 0.5, q, MULT, ADD)
            tanh_s = gate_pool.tile([BL, M], BF16, tag="tanhs")
            nc.scalar.activation(tanh_s, s_nat, TANH, scale=0.5)
            nc.vector.scalar_tensor_tensor(h_nat, t_o, 1.0, tanh_s, ADD, MULT)

            # ---- transpose new h, s back to ^T layout -------------------
            for c in range(2):
                trh = ps_tr.tile([128, BL], BF16, tag="trh")
                nc.tensor.transpose(trh, h_nat[:, c * 128:(c + 1) * 128], id_s)
                nc.vector.tensor_copy(h_bf[:, c, :], trh)
                trs = ps_tr.tile([128, BL], BF16, tag="trs")
                nc.tensor.transpose(trs, s_nat[:, c * 128:(c + 1) * 128], id_s)
                nc.vector.tensor_copy(s_bf[:, c, :], trs)

            # ---- attention energies + softmax ---------------------------
            e_ps = ps_e.tile([BL, N], F32, tag="eps")
            first = True
            for half in range(2):
                for c in range(NCHUNK):
                    tin = work.tile([128, BCHUNK * N], BF16, tag="tin")
                    for bb in range(BCHUNK):
                        b = c * BCHUNK + bb
                        nc.vector.tensor_scalar(
                            out=tin[:, bb * N:(bb + 1) * N],
                            in0=r2T[:, half, b, :],
                            scalar1=r1_sb[:, half, b:b + 1],
                            scalar2=None,
                            op0=ADD,
                        )
                    tout = work.tile([128, BCHUNK * N], BF16, tag="tout")
                    nc.scalar.activation(tout, tin, TANH)
                    for bb in range(BCHUNK):
                        b = c * BCHUNK + bb
                        last = (half == 1 and c == NCHUNK - 1 and bb == BCHUNK - 1)
                        nc.tensor.matmul(
                            e_ps,
                            lhsT=vs_s[:, half, b, :],
                            rhs=tout[:, bb * N:(bb + 1) * N],
                            start=first,
                            stop=last,
                        )
                        first = False

            exp_sb = opool.tile([BL, N], BF16, tag="expsb")
            esum = opool.tile([BL, 1], F32, tag="esum")
            nc.scalar.activation(exp_sb, e_ps, EXP, accum_out=esum)
            rsum = opool.tile([BL, 1], F32, tag="rsum")
            nc.vector.reciprocal(rsum, esum)
            outv = opool.tile([BL, N], F32, tag="outv")
            nc.vector.scalar_tensor_tensor(outv, exp_sb, rsum, x_t_sb,
                                           MULT, MULT)
            nc.sync.dma_start(out=out_p[:, t, :], in_=outv)

    nc.compile()
    return nc


def _marshal(x, s, h, We, Ue, ve, Wk, Wr, b):
    """Host-side input prep (sharding + weight prepacking, no x-dependent math)."""
    bf = ml_dtypes.bfloat16
    x_bf = x.astype(bf)                                   # [B, T, N]
    xt_bf = np.ascontiguousarray(x_bf.transpose(1, 0, 2)) # [T, B, N]
    h2 = (h.astype(np.float32) * 2.0)   # doubled states
    s2 = (s.astype(np.float32) * 2.0)
    hT = np.ascontiguousarray(h2.astype(bf).T)            # [M, B]
    sT = np.ascontiguousarray(s2.astype(bf).T)

    ue_w = np.ascontiguousarray(Ue.astype(bf).reshape(2, 128, T))
    we_w = np.ascontiguousarray(
        (We.astype(np.float32) * 0.5).astype(bf).reshape(4, 128, T))
    wc = np.concatenate([Wk, Wr * 0.5], axis=0).astype(np.float32)  # [N+M, 4M]
    wc[:, 2 * M:3 * M] *= 2.0    # pre-scale g gate so tanh uses scale=0.5
    wc_w = np.ascontiguousarray(wc.astype(bf).reshape(3, 128, M4))

    vs = np.zeros((128, 2, BL, BL), dtype=bf)
    vef = ve[:, 0].astype(np.float32)
    for half in range(2):
        seg = vef[half * 128:(half + 1) * 128].astype(bf)
        for bb in range(BL):
            vs[:, half, bb, bb] = seg

    ue_blob = ue_w.transpose(1, 0, 2).reshape(128, -1)
    we_blob = we_w.transpose(1, 0, 2).reshape(128, -1)
    wc_blob = wc_w.transpose(1, 0, 2).reshape(128, -1)
    vs_blob = vs.reshape(128, -1)
    id64 = np.eye(BL, dtype=bf)

    with_bias = bool(np.any(b))
    bias2 = b.astype(np.float32).copy()
    bias2[2 * M:3 * M] *= 2.0
    bias_nat = np.ascontiguousarray(
        np.broadcast_to(bias2, (BL, M4)).astype(np.float32))

    in_maps = []
    for i in range(NCORES):
        sl = slice(i * BL, (i + 1) * BL)
        xt_core = xt_bf[:, sl, :].reshape(2, 128, BL, N)
        blob = np.concatenate([
            xt_core.transpose(1, 0, 2, 3).reshape(128, -1),
            ue_blob, we_blob, wc_blob, vs_blob,
        ], axis=1)
        m = {
            "x_b": np.ascontiguousarray(x_bf[sl]),
            "x_n": np.ascontiguousarray(x_bf[sl].transpose(1, 2, 0)),
            "blob": np.ascontiguousarray(blob),
            "hT0": np.ascontiguousarray(hT[:, sl].reshape(2, 128, BL)),
            "sT0": np.ascontiguousarray(sT[:, sl].reshape(2, 128, BL)),
            "hn0": np.ascontiguousarray(h2[sl].astype(bf)),
            "sn0": np.ascontiguousarray(s2[sl].astype(bf)),
            "id64": id64,
        }
        if with_bias:
            m["biasn"] = bias_nat
        in_maps.append(m)
    return in_maps, with_bias


def kernel(**inputs) -> np.ndarray:
    x = np.asarray(inputs["x"])
    s = np.asarray(inputs["s"])
    h = np.asarray(inputs["h"])
    We = np.asarray(inputs["We"])
    Ue = np.asarray(inputs["Ue"])
    ve = np.asarray(inputs["ve"])
    Wk = np.asarray(inputs["Wk"])
    Wr = np.asarray(inputs["Wr"])
    b = np.asarray(inputs["b"])

    in_maps, with_bias = _marshal(x, s, h, We, Ue, ve, Wk, Wr, b)
    nc = build_nc(T, with_bias=with_bias)
    res = run_bass_kernel_spmd(nc, in_maps, core_ids=list(range(NCORES)))
    out = np.concatenate([r["out"] for r in res.results], axis=0)
    return out.astype(np.float32)


if __name__ == "__main__":
    rng = np.random.default_rng(0)
    demo = {
        "x": rng.standard_normal((B, T, N), dtype=np.float32),
        "s": rng.standard_normal((B, M), dtype=np.float32) * 0.1,
        "h": rng.standard_normal((B, M), dtype=np.float32) * 0.1,
        "We": rng.standard_normal((2 * M, T), dtype=np.float32) / np.sqrt(2 * M),
        "Ue": rng.standard_normal((T, T), dtype=np.float32) / np.sqrt(T),
        "ve": rng.standard_normal((T, 1), dtype=np.float32) / np.sqrt(T),
        "Wk": rng.standard_normal((N, M4), dtype=np.float32) / np.sqrt(N),
        "Wr": rng.standard_normal((M, M4), dtype=np.float32) / np.sqrt(M),
        "b": np.zeros((M4,), dtype=np.float32),
    }
    out = kernel(**demo)
    print(out.shape, out.dtype)


# revision 18
# speedup vs baseline: 12.5903x; 1.9233x over previous
"""Trainium2 Bass kernel for the attention-encoder (Bahdanau input attention
+ LSTM cell, T-step recurrence).

Math (per batch row b):
    r2 = einsum('tn,tu->nu', x[b], Ue)                 # [N, T], loop-invariant
    per step t:
        r1 = concat(h, s) @ We                         # [T]
        e[n] = sum_t' ve[t'] * tanh(r1[t'] + r2[n,t']) # [N]
        alpha = softmax_n(e)
        z = x_t @ Wk + h @ Wr + b ; LSTM update (keras gate order i,f,g,o)
        out[b, t, :] = alpha * x[b, t, :]

Strategy: pure data parallelism, batch 512 -> 64 per core on 8 cores.
On-chip layout keeps t' on partitions for the big pass:
    r2T [t'(2x128 part), b, n]  (bf16)
    per step: DVE tensor_scalar adds r1[b,t'] (per-partition scalar),
    ACT does one big tanh per chunk, PE contracts t' against a
    per-b "selector" stationary (col b = ve-half) accumulating
    e into PSUM[b, n] -- natural layout for the free-axis softmax.
LSTM computes z in natural layout ([b, 4M]) with stationaries x_t^T/h^T,
one fused gate tanh (g-gate weights pre-scaled x2 on host so all gates
share scale=0.5), sigmoid-as-tanh to stay in the exp/tanh ACT table set,
then PE-transposes h/s back to the ^T layout the r1/z matmuls need.
"""

import numpy as np
import ml_dtypes
from contextlib import ExitStack

import concourse.bass as bass
import concourse.bacc as bacc
import concourse.tile as tile
from concourse import mybir
from concourse.bass_utils import run_bass_kernel_spmd

B, T, N, M = 512, 256, 128, 256
NCORES = 8
BL = B // NCORES  # 64 batch rows per core
M4 = 4 * M        # 1024

BF16 = mybir.dt.bfloat16
F32 = mybir.dt.float32
TANH = mybir.ActivationFunctionType.Tanh
EXP = mybir.ActivationFunctionType.Exp
ADD = mybir.AluOpType.add
MULT = mybir.AluOpType.mult

BCHUNK = 32             # b-rows per attention chunk (free = BCHUNK*N = 4096)
NCHUNK = BL // BCHUNK   # chunks per t'-half

# blob free-dim offsets (all [128, *] bf16, packed on host by _marshal)
OFF_XT = 0                       # x_tmaj  [p, 2, BL, N]
OFF_UE = OFF_XT + 2 * BL * N     # Ue      [p, 2, T]
OFF_WE = OFF_UE + 2 * T          # We      [p, 4, T]
OFF_WC = OFF_WE + 4 * T          # Wc      [p, 3, M4]  (g cols pre-scaled x2)
OFF_VS = OFF_WC + 3 * M4         # vsel    [p, 2, BL, BL]
BLOB_F = OFF_VS + 2 * BL * BL


def build_nc(t_steps: int = T, with_bias: bool = False,
             repeats: int = 1) -> bass.Bass:
    nc = bacc.Bacc(None)

    x_p = nc.declare_dram_parameter("x_b", [BL, T, N], BF16, isOutput=False)
    xn_p = nc.declare_dram_parameter("x_n", [T, N, BL], BF16, isOutput=False)
    blob_p = nc.declare_dram_parameter("blob", [128, BLOB_F], BF16, isOutput=False)
    hT_p = nc.declare_dram_parameter("hT0", [2, 128, BL], BF16, isOutput=False)
    sT_p = nc.declare_dram_parameter("sT0", [2, 128, BL], BF16, isOutput=False)
    hn_p = nc.declare_dram_parameter("hn0", [BL, M], BF16, isOutput=False)
    sn_p = nc.declare_dram_parameter("sn0", [BL, M], BF16, isOutput=False)
    id_p = nc.declare_dram_parameter("id64", [BL, BL], BF16, isOutput=False)
    if with_bias:
        bb_p = nc.declare_dram_parameter("biasn", [BL, M4], F32, isOutput=False)
    out_p = nc.declare_dram_parameter("out", [BL, T, N], F32, isOutput=True)

    with tile.TileContext(nc) as tc, ExitStack() as ctx:
        singles = ctx.enter_context(tc.tile_pool(name="singles", bufs=1))

        # ---- resident tensors -------------------------------------------
        blob = singles.tile([128, BLOB_F], BF16)
        r2T = singles.tile([128, 2, BL, N], BF16)      # r2[t', b, n]
        h_bf = singles.tile([128, 2, BL], BF16)        # h^T state
        s_bf = singles.tile([128, 2, BL], BF16)        # s^T state
        h_nat = singles.tile([BL, M], BF16)            # h natural state
        s_nat = singles.tile([BL, M], BF16)            # s natural state
        id_s = singles.tile([BL, BL], BF16)            # 64x64 identity
        if with_bias:
            bb_s = singles.tile([BL, M4], F32)

        x_tmaj = blob[:, OFF_XT:OFF_UE].rearrange(
            "p (h b n) -> p h b n", h=2, b=BL)
        ue_s = blob[:, OFF_UE:OFF_WE].rearrange("p (h t) -> p h t", h=2)
        we_s = blob[:, OFF_WE:OFF_WC].rearrange("p (j t) -> p j t", j=4)
        wc_s = blob[:, OFF_WC:OFF_VS].rearrange("p (j m) -> p j m", j=3)
        vs_s = blob[:, OFF_VS:BLOB_F].rearrange(
            "p (h b m) -> p h b m", h=2, b=BL)

        nc.sync.dma_start(out=blob, in_=blob_p[:])
        nc.sync.dma_start(out=h_bf, in_=hT_p.rearrange("h p b -> p h b"))
        nc.sync.dma_start(out=s_bf, in_=sT_p.rearrange("h p b -> p h b"))
        nc.sync.dma_start(out=h_nat, in_=hn_p[:])
        nc.sync.dma_start(out=s_nat, in_=sn_p[:])
        nc.sync.dma_start(out=id_s, in_=id_p[:])
        if with_bias:
            nc.sync.dma_start(out=bb_s, in_=bb_p[:])

        # ---- precompute r2T: r2[t',b,n] = sum_t Ue[t,t'] x[b,t,n] --------
        # moving spans 4 b-blocks (FD=512, one PSUM bank) per matmul
        with tc.tile_pool(name="pre_ps", bufs=8, space="PSUM") as pre_ps:
            for c in range(2):          # t'-half (output partitions)
                for g in range(BL // 4):
                    r2p = pre_ps.tile([128, 4 * N], F32, tag="r2p")
                    for k in range(2):  # contraction half
                        nc.tensor.matmul(
                            r2p,
                            lhsT=ue_s[:, k, c * 128:(c + 1) * 128],
                            rhs=x_tmaj[:, k, 4 * g:4 * g + 4, :].rearrange(
                                "p b n -> p (b n)"),
                            start=(k == 0),
                            stop=(k == 1),
                        )
                    dst = r2T[:, c, 4 * g:4 * g + 4, :].rearrange(
                        "p b n -> p (b n)")
                    if g % 2 == 0:
                        nc.vector.tensor_copy(dst, r2p)
                    else:
                        nc.scalar.copy(dst, r2p)

        # ---- per-step pools ---------------------------------------------
        work = ctx.enter_context(tc.tile_pool(name="work", bufs=3))
        gate_pool = ctx.enter_context(tc.tile_pool(name="gates", bufs=2))
        ps_z = ctx.enter_context(tc.tile_pool(name="ps_z", bufs=1, space="PSUM"))
        ps_r1 = ctx.enter_context(tc.tile_pool(name="ps_r1", bufs=1, space="PSUM"))
        ps_e = ctx.enter_context(tc.tile_pool(name="ps_e", bufs=2, space="PSUM"))
        ps_tr = ctx.enter_context(tc.tile_pool(name="ps_tr", bufs=1, space="PSUM"))
        xfeed = ctx.enter_context(tc.tile_pool(name="xfeed", bufs=3))
        opool = ctx.enter_context(tc.tile_pool(name="opool", bufs=3))

        def fetch_x(t):
            x_t_sb = xfeed.tile([BL, N], BF16, tag="x_t")
            nc.sync.dma_start(out=x_t_sb, in_=x_p[:, t, :])
            x_tT_sb = xfeed.tile([128, BL], BF16, tag="x_tT")
            nc.sync.dma_start(out=x_tT_sb, in_=xn_p[t])
            return x_t_sb, x_tT_sb

        x_feed = fetch_x(0)

        for t in [tt for _ in range(repeats) for tt in range(t_steps)]:
            x_t_sb, x_tT_sb = x_feed
            if t + 1 < t_steps:
                x_feed = fetch_x(t + 1)

            # ---- r1^T = We^T @ [h; s]  -> [t'(2x128), b] ----------------
            r1_ps = ps_r1.tile([128, 2, BL], F32, tag="r1ps")
            for c in range(2):
                for j in range(4):
                    rhs = h_bf[:, j, :] if j < 2 else s_bf[:, j - 2, :]
                    nc.tensor.matmul(
                        r1_ps[:, c, :],
                        lhsT=we_s[:, j, c * 128:(c + 1) * 128],
                        rhs=rhs,
                        start=(j == 0),
                        stop=(j == 3),
                    )
            r1_sb = work.tile([128, 2, BL], F32, tag="r1sb")
            nc.vector.tensor_copy(r1_sb, r1_ps)

            # ---- z natural: [b, 4M] = x_t @ Wk + h @ Wr -----------------
            # stationary = x_tT / hT (k on partitions, cols = b),
            # moving = weight blocks; 6 matmuls of FD=512.
            z_ps = ps_z.tile([BL, M4], F32, tag="zps")
            for mh in range(2):
                sl = slice(mh * 512, (mh + 1) * 512)
                for j in range(3):
                    lhsT = x_tT_sb if j == 0 else h_bf[:, j - 1, :]
                    nc.tensor.matmul(
                        z_ps[:, sl],
                        lhsT=lhsT,
                        rhs=wc_s[:, j, sl],
                        start=(j == 0),
                        stop=(j == 2),
                    )
            if with_bias:
                nc.vector.tensor_add(z_ps, z_ps, bb_s)

            # ---- gates: one fused tanh(0.5 z) over all 4 gates ----------
            t_all = gate_pool.tile([BL, M4], BF16, tag="tall")
            nc.scalar.activation(t_all, z_ps, TANH, scale=0.5)
            t_i = t_all[:, 0:M]
            t_f = t_all[:, M:2 * M]
            t_g = t_all[:, 2 * M:3 * M]   # = tanh(z_g) via host 2x prescale
            t_o = t_all[:, 3 * M:M4]

            # states are doubled (H=2h, S=2s; the 0.5 is folded into the
            # We/Wr weight rows on the host):
            #   S_new = 0.5*(t_f+1)*S + (t_i+1)*t_g
            #   H_new = (t_o+1)*tanh(0.5*S_new)
            v = gate_pool.tile([BL, M], BF16, tag="v")
            nc.vector.scalar_tensor_tensor(v, t_f, 1.0, s_nat, ADD, MULT)
            q = gate_pool.tile([BL, M], BF16, tag="q")
            nc.vector.scalar_tensor_tensor(q, t_i, 1.0, t_g, ADD, MULT)
            nc.vector.scalar_tensor_tensor(s_nat, v, 0.5, q, MULT, ADD)
            tanh_s = gate_pool.tile([BL, M], BF16, tag="tanhs")
            nc.scalar.activation(tanh_s, s_nat, TANH, scale=0.5)
            nc.vector.scalar_tensor_tensor(h_nat, t_o, 1.0, tanh_s, ADD, MULT)

            # ---- transpose new h, s back to ^T layout -------------------
            for c in range(2):
                trh = ps_tr.tile([128, BL], BF16, tag="trh")
                nc.tensor.transpose(trh, h_nat[:, c * 128:(c + 1) * 128], id_s)
                nc.vector.tensor_copy(h_bf[:, c, :], trh)
                trs = ps_tr.tile([128, BL], BF16, tag="trs")
                nc.tensor.transpose(trs, s_nat[:, c * 128:(c + 1) * 128], id_s)
                nc.vector.tensor_copy(s_bf[:, c, :], trs)

            # ---- attention energies + softmax ---------------------------
            e_ps = ps_e.tile([BL, N], F32, tag="eps")
            first = True
            for half in range(2):
                for c in range(NCHUNK):
                    tin = work.tile([128, BCHUNK * N], BF16, tag="tin")
                    for bb in range(BCHUNK):
                        b = c * BCHUNK + bb
                        nc.vector.tensor_scalar(
                            out=tin[:, bb * N:(bb + 1) * N],
                            in0=r2T[:, half, b, :],
                            scalar1=r1_sb[:, half, b:b + 1],
                            scalar2=None,
                            op0=ADD,
                        )
                    tout = work.tile([128, BCHUNK * N], BF16, tag="tout")
                    nc.scalar.activation(tout, tin, TANH)
                    for bb in range(BCHUNK):
                        b = c * BCHUNK + bb
                        last = (half == 1 and c == NCHUNK - 1 and bb == BCHUNK - 1)
                        nc.tensor.matmul(
                            e_ps,
                            lhsT=vs_s[:, half, b, :],
                            rhs=tout[:, bb * N:(bb + 1) * N],
                            start=first,
                            stop=last,
                        )
                        first = False

            exp_sb = opool.tile([BL, N], BF16, tag="expsb")
            esum = opool.tile([BL, 1], F32, tag="esum")
            nc.scalar.activation(exp_sb, e_ps, EXP, accum_out=esum)
            rsum = opool.tile([BL, 1], F32, tag="rsum")
            nc.vector.reciprocal(rsum, esum)
            outv = opool.tile([BL, N], F32, tag="outv")
            nc.vector.scalar_tensor_tensor(outv, exp_sb, rsum, x_t_sb,
                                           MULT, MULT)
            nc.sync.dma_start(out=out_p[:, t, :], in_=outv)

    nc.compile()
    return nc


def _marshal(x, s, h, We, Ue, ve, Wk, Wr, b):
    """Host-side input prep (sharding + weight prepacking, no x-dependent math)."""
    bf = ml_dtypes.bfloat16
    x_bf = x.astype(bf)                                   # [B, T, N]
    xt_bf = np.ascontiguousarray(x_bf.transpose(1, 0, 2)) # [T, B, N]
    h2 = (h.astype(np.float32) * 2.0)   # doubled states
    s2 = (s.astype(np.float32) * 2.0)
    hT = np.ascontiguousarray(h2.astype(bf).T)            # [M, B]
    sT = np.ascontiguousarray(s2.astype(bf).T)

    ue_w = np.ascontiguousarray(Ue.astype(bf).reshape(2, 128, T))
    we_w = np.ascontiguousarray(
        (We.astype(np.float32) * 0.5).astype(bf).reshape(4, 128, T))
    wc = np.concatenate([Wk, Wr * 0.5], axis=0).astype(np.float32)  # [N+M, 4M]
    wc[:, 2 * M:3 * M] *= 2.0    # pre-scale g gate so tanh uses scale=0.5
    wc_w = np.ascontiguousarray(wc.astype(bf).reshape(3, 128, M4))

    vs = np.zeros((128, 2, BL, BL), dtype=bf)
    vef = ve[:, 0].astype(np.float32)
    for half in range(2):
        seg = vef[half * 128:(half + 1) * 128].astype(bf)
        for bb in range(BL):
            vs[:, half, bb, bb] = seg

    ue_blob = ue_w.transpose(1, 0, 2).reshape(128, -1)
    we_blob = we_w.transpose(1, 0, 2).reshape(128, -1)
    wc_blob = wc_w.transpose(1, 0, 2).reshape(128, -1)
    vs_blob = vs.reshape(128, -1)
    id64 = np.eye(BL, dtype=bf)

    with_bias = bool(np.any(b))
    bias2 = b.astype(np.float32).copy()
    bias2[2 * M:3 * M] *= 2.0
    bias_nat = np.ascontiguousarray(
        np.broadcast_to(bias2, (BL, M4)).astype(np.float32))

    in_maps = []
    for i in range(NCORES):
        sl = slice(i * BL, (i + 1) * BL)
        xt_core = xt_bf[:, sl, :].reshape(2, 128, BL, N)
        blob = np.concatenate([
            xt_core.transpose(1, 0, 2, 3).reshape(128, -1),
            ue_blob, we_blob, wc_blob, vs_blob,
        ], axis=1)
        m = {
            "x_b": np.ascontiguousarray(x_bf[sl]),
            "x_n": np.ascontiguousarray(x_bf[sl].transpose(1, 2, 0)),
            "blob": np.ascontiguousarray(blob),
            "hT0": np.ascontiguousarray(hT[:, sl].reshape(2, 128, BL)),
            "sT0": np.ascontiguousarray(sT[:, sl].reshape(2, 128, BL)),
            "hn0": np.ascontiguousarray(h2[sl].astype(bf)),
            "sn0": np.ascontiguousarray(s2[sl].astype(bf)),
            "id64": id64,
        }
        if with_bias:
            m["biasn"] = bias_nat
        in_maps.append(m)
    return in_maps, with_bias


def kernel(**inputs) -> np.ndarray:
    x = np.asarray(inputs["x"])
    s = np.asarray(inputs["s"])
    h = np.asarray(inputs["h"])
    We = np.asarray(inputs["We"])
    Ue = np.asarray(inputs["Ue"])
    ve = np.asarray(inputs["ve"])
    Wk = np.asarray(inputs["Wk"])
    Wr = np.asarray(inputs["Wr"])
    b = np.asarray(inputs["b"])

    in_maps, with_bias = _marshal(x, s, h, We, Ue, ve, Wk, Wr, b)
    nc = build_nc(T, with_bias=with_bias)
    res = run_bass_kernel_spmd(nc, in_maps, core_ids=list(range(NCORES)))
    out = np.concatenate([r["out"] for r in res.results], axis=0)
    return out.astype(np.float32)


if __name__ == "__main__":
    rng = np.random.default_rng(0)
    demo = {
        "x": rng.standard_normal((B, T, N), dtype=np.float32),
        "s": rng.standard_normal((B, M), dtype=np.float32) * 0.1,
        "h": rng.standard_normal((B, M), dtype=np.float32) * 0.1,
        "We": rng.standard_normal((2 * M, T), dtype=np.float32) / np.sqrt(2 * M),
        "Ue": rng.standard_normal((T, T), dtype=np.float32) / np.sqrt(T),
        "ve": rng.standard_normal((T, 1), dtype=np.float32) / np.sqrt(T),
        "Wk": rng.standard_normal((N, M4), dtype=np.float32) / np.sqrt(N),
        "Wr": rng.standard_normal((M, M4), dtype=np.float32) / np.sqrt(M),
        "b": np.zeros((M4,), dtype=np.float32),
    }
    out = kernel(**demo)
    print(out.shape, out.dtype)
